# revision 1
# baseline (speedup 1.0000x reference)
"""Trainium2 Bass kernel for nn_EquivariantAttention (GNN message passing).

Strategy
--------
* Host: sort edges by destination node; split the 10000 nodes into 8
  contiguous ranges of 1250 (edge counts are ~E/8 each for the uniform
  graph); each NeuronCore owns one node range and all edges pointing into
  it -> no cross-core communication at all.
* Per core the edges are packed into a (128 partitions x T slots) layout
  such that every destination-node segment lives inside one partition
  (host pads partitions with dummy slots).  Slot (p, t) <-> flat index
  j = t*128 + p, which is exactly the layout dma_gather produces.
* PE (fp32r) does the radial-MLP matmuls:
    h = gelu(w1^T @ efT + b)            (feature-major, 128 x E)
    [left|right] = h_tile^T @ [wl|wr]   (edge-major: stationary = h tile)
  plus bias via rank-1 accumulate matmuls.
* DVE does the per-edge small bilinear contractions (basis contract,
  low-rank apply) as multiply + grouped tensor_reduce with free-dim
  broadcast access patterns.
* Segment softmax: scores are stored (head, t) per partition;
  segment max via masked forward scan + reverse broadcast scan
  (tensor_tensor_scan with negative-stride APs); exp on ACT; the
  segment sums (denominator and weighted V) are done with
  dma_scatter_add (CCE add) into DRAM keyed by destination node.
* Final: node-major readback, out = wv_sum * 1/den, DMA out.

kernel(**inputs) takes the full-problem arrays and returns (10000,16,4).
"""

import os
import sys
import numpy as np

for _p in ("/opt/trn_rl_repo", "/root/.axon_site/_ro/trn_rl_repo"):
    if os.path.isdir(_p) and _p not in sys.path:
        sys.path.insert(0, _p)

# --- static problem config (matches reference.py) ---
N = 10000
E = 160000
MULT = 16
NL = 2
DIM = 4
EDGE_DIM = 32
HID = 128
RANK = 8
H = 4
HM = MULT // H
NCORES = 8
NPC = N // NCORES          # nodes per core = 1250
NCP = 1280                 # padded local node rows (multiple of 128)
NCH = NCP // 128           # node chunks = 10
QROWS = NCP + 128          # + junk/zero chunk (junk row index = NCP)

NEG = -3.0e38
GELU_MODE = "hw"   # "hw": ACT Gelu LUT; "sim": tanh-approx composite
DEBUG = False


# ----------------------------------------------------------------------
# Host-side prep
# ----------------------------------------------------------------------

def _pack_core(dst_sorted_local, T):
    """Greedy-pack node segments (sorted local dst ids) into 128 partitions
    with T slots each.  Returns (slot_of_edge (Ec,), node_slot_info) or None
    if T is too small.  dst_sorted_local: local dst id per edge, sorted."""
    Ec = len(dst_sorted_local)
    # segment boundaries
    nodes, starts = np.unique(dst_sorted_local, return_index=True)
    ends = np.append(starts[1:], Ec)
    p = 0
    fill = 0
    seg_part = np.empty(len(nodes), np.int32)
    seg_off = np.empty(len(nodes), np.int32)
    for s in range(len(nodes)):
        ln = ends[s] - starts[s]
        if ln > T:
            return None
        if fill + ln > T:
            p += 1
            fill = 0
            if p >= 128:
                return None
        seg_part[s] = p
        seg_off[s] = fill
        fill += ln
    # slot index per edge
    slot_p = np.empty(Ec, np.int32)
    slot_t = np.empty(Ec, np.int32)
    for s in range(len(nodes)):
        a, b = starts[s], ends[s]
        slot_p[a:b] = seg_part[s]
        slot_t[a:b] = seg_off[s] + np.arange(b - a)
    return slot_p, slot_t, nodes, starts, ends, seg_part, seg_off


def _prep_core(order_c, dst_c, src_c, b1, b2, ef, n0, T):
    """Build all per-core host arrays for one core. order_c: indices into the
    full edge arrays (already sorted by dst)."""
    Ec = len(order_c)
    dst_local = dst_c - n0
    packed = _pack_core(dst_local, T)
    assert packed is not None
    slot_p, slot_t, nodes, starts, ends, seg_part, seg_off = packed

    ES = 128 * T
    # edge-major slot arrays
    b1EM = np.zeros((128, T, 8), np.float32)
    b2EM = np.zeros((128, T, 8), np.float32)
    efT = np.zeros((32, ES), np.float32)
    isrc = np.full(ES, N, np.int64)        # pad -> zero row of f table
    idst = np.full(ES, NCP, np.int64)      # pad -> zero row of q table
    iout = np.full(ES, NCP, np.int64)      # non-end/pad -> junk row

    flat = slot_t * 128 + slot_p
    b1EM[slot_p, slot_t] = b1[order_c].reshape(Ec, 8)
    b2EM[slot_p, slot_t] = b2[order_c].reshape(Ec, 8)
    efT[:, flat] = ef[order_c].T
    isrc[flat] = src_c
    idst[flat] = dst_local
    # scatter index: ONLY the segment-end slot carries the node id (the
    # running-sum scan puts the segment total there); everything else goes
    # to the junk row, so every real output row is written exactly once
    # (dma_scatter_add races on duplicate destinations on HW).
    end_flat = (seg_off + (ends - starts) - 1) * 128 + seg_part
    iout[end_flat] = nodes

    # masks in (p, t): start of segment / end of segment / pad
    start = np.zeros((128, T), np.float32)
    endf = np.zeros((128, T), np.float32)
    pad = np.zeros((128, T), np.float32)
    pad[slot_p, slot_t] = 1.0
    for s in range(len(nodes)):
        start[seg_part[s], seg_off[s]] = 1.0
        endf[seg_part[s], seg_off[s] + (ends[s] - starts[s]) - 1] = 1.0
    # every pad slot is its own segment start (keeps scans bounded)
    start = np.maximum(start, 1.0 - pad)

    def rep4(m):  # (128,T) -> (128, 4*T) head-major
        return np.tile(m[:, None, :], (1, 4, 1)).reshape(128, 4 * T)

    masks = dict(
        sneg=rep4(start * NEG),          # add-reset mask for max scan
        skeep=rep4(1.0 - start),         # mult-keep mask for sum scan (unused now)
        ekeep=rep4((1.0 - endf) * pad),  # reverse-scan keep (0 at seg end & pads)
        esel=rep4(endf),                 # select segment ends
        pad=rep4(pad),
    )

    def wrap16(ix):
        w = ix.reshape(-1, 16).T.astype(np.int16)   # (16, ES//16)
        return np.ascontiguousarray(np.tile(w, (8, 1)))  # replicate to 128

    return dict(
        b1EM=b1EM.reshape(128, T * 8),
        b2EM=b2EM.reshape(128, T * 8),
        efT=efT,
        isrc=wrap16(isrc),
        idst=wrap16(idst),
        iout=wrap16(iout),
        **{"m_" + k: v for k, v in masks.items()},
    )


def _host_prep(b1, b2, edge_feats, f, src_idx, dst_idx,
               q_w, q_b, kv_params):
    src = np.asarray(src_idx).astype(np.int64)
    dst = np.asarray(dst_idx).astype(np.int64)
    order = np.argsort(dst, kind="stable")
    dst_s = dst[order]

    # per-core edge ranges (node split at multiples of NPC)
    bounds = np.searchsorted(dst_s, np.arange(0, N + 1, NPC))
    core_edges = [order[bounds[i]:bounds[i + 1]] for i in range(NCORES)]

    # find a common T
    T = int(max(2, -(-max(len(c) for c in core_edges) // 128)))
    while True:
        T = -(-T // 8) * 8
        ok = True
        for i in range(NCORES):
            dl = dst_s[bounds[i]:bounds[i + 1]] - i * NPC
            if _pack_core(dl, T) is None:
                ok = False
                break
        if ok:
            break
        T += 8

    # shared (per-core identical) weight arrays
    f64 = np.asarray(f, np.float32).reshape(N, 64)
    fG = np.concatenate([f64, np.zeros((1, 64), np.float32)], 0)  # zero row N

    # q: W = q_w.reshape(NL,MULT,MULT)[IDX]; q[n,o,d] = sum_m W[d,o,m] f[n,m,d]
    IDX = np.array([0, 1, 1, 1])
    Wn = np.asarray(q_w, np.float32).reshape(NL, MULT, MULT)
    Wbig = np.zeros((128, 64), np.float32)
    for d in range(DIM):
        Wd = Wn[IDX[d]]                      # (o, m)
        for m in range(MULT):
            for o in range(MULT):
                Wbig[d * 32 + m, o * 4 + d] = Wd[o, m] * 0.25  # fold 1/sqrt(HM*DIM)
    qbRow = np.zeros((1, 64), np.float32)
    qbRow[0, 0::4] = np.asarray(q_b, np.float32)[:, 0] * 0.25

    shared = dict(Wq=Wbig, qbRow=qbRow, fG=fG,
                  ident=np.eye(64, dtype=np.float32),
                  ones512=np.ones((1, 512), np.float32))
    for p in ("k", "v"):
        w1, bb1, wl, bl, wr, br = kv_params[p]
        shared[p + "w1"] = np.asarray(w1, np.float32)              # (32, 128)
        shared[p + "WLR"] = np.concatenate(
            [np.asarray(wl, np.float32), np.asarray(wr, np.float32)], 1)  # (128,512)
        shared[p + "bRow"] = np.concatenate(
            [np.asarray(bl, np.float32), np.asarray(br, np.float32)])[None, :]
        shared[p + "gb"] = np.asarray(bb1, np.float32)[:, None]    # (128,1)

    in_maps = []
    for i in range(NCORES):
        oc = core_edges[i]
        m = _prep_core(oc, dst[oc], src[oc],
                       np.asarray(b1, np.float32), np.asarray(b2, np.float32),
                       np.asarray(edge_feats, np.float32), i * NPC, T)
        # per-core f slice for q matmul: partition d*32+m, col = local node
        fT = np.zeros((128, NCP), np.float32)
        fl = np.asarray(f, np.float32)[i * NPC:(i + 1) * NPC]  # (NPC, 16, 4)
        for d in range(DIM):
            fT[d * 32:d * 32 + 16, :NPC] = fl[:, :, d].T
        m["fTq"] = fT
        m.update(shared)
        in_maps.append(m)
    return in_maps, T


# ----------------------------------------------------------------------
# Bass program
# ----------------------------------------------------------------------

def build_program(T):
    import concourse.bass as bass
    import concourse.tile as tile
    from concourse import bacc as bacc_mod
    from concourse import mybir

    dt = mybir.dt
    f32, i16 = dt.float32, dt.int16
    f32r = dt.float32r
    Alu = mybir.AluOpType
    Act = mybir.ActivationFunctionType
    AxX = mybir.AxisListType.X
    ES = 128 * T
    CH = T // 8
    T4 = 4 * T

    nc = bacc_mod.Bacc()

    def din(name, shape, dtype=f32):
        return nc.declare_dram_parameter(name, list(shape), dtype, isOutput=False)

    # inputs
    efT_d = din("efT", (32, ES), f32r)
    b1_d = din("b1EM", (128, T * 8))
    b2_d = din("b2EM", (128, T * 8))
    fG_d = din("fG", (N + 1, 64))
    fTq_d = din("fTq", (128, NCP))
    isrc_d = din("isrc", (128, ES // 16), i16)
    idst_d = din("idst", (128, ES // 16), i16)
    iout_d = din("iout", (128, ES // 16), i16)
    Wq_d = din("Wq", (128, 64))
    qbRow_d = din("qbRow", (1, 64))
    wparams = {}
    for p in ("k", "v"):
        wparams[p] = (din(p + "w1", (32, HID), f32r),
                      din(p + "WLR", (HID, 512), f32r),
                      din(p + "bRow", (1, 512), f32r), din(p + "gb", (HID, 1)))
    masks_d = {k: din("m_" + k, (128, T4)) for k in
               ("sneg", "skeep", "ekeep", "esel", "pad")}

    out_d = nc.declare_dram_parameter("out", [NCP, 64], f32, isOutput=True)
    q_dram = nc.dram_tensor("q_dram", [QROWS, 64], f32)
    wv_dram = nc.dram_tensor("wv_dram", [QROWS, 64], f32)
    den_dram = nc.dram_tensor("den_dram", [QROWS, 64], f32)

    ident_d = din("ident", (64, 64))
    ones_d = din("ones512", (1, 512), f32r)

    def AP(t, offset, ap):
        return bass.AP(tensor=t.tensor, offset=t.offset + offset, ap=ap)


    from contextlib import ExitStack
    with tile.TileContext(nc) as tc, ExitStack() as st:
        cst = st.enter_context(tc.tile_pool(name="cst", bufs=1))
        big = st.enter_context(tc.tile_pool(name="big", bufs=1))
        ld = st.enter_context(tc.tile_pool(name="ld", bufs=2))
        sm = st.enter_context(tc.tile_pool(name="sm", bufs=2))
        ps = st.enter_context(tc.tile_pool(name="ps", bufs=2, space="PSUM"))
        psq = st.enter_context(tc.tile_pool(name="psq", bufs=1, space="PSUM"))
        if True:
            # ---- constants to SBUF
            def load(d, shape, dtype=f32):
                t = cst.tile(list(shape), dtype, name="c_" + d.name)
                nc.sync.dma_start(out=t[:], in_=d[:])
                return t

            sb_w = {}
            for p in ("k", "v"):
                w1, WLR, bRow, gb = wparams[p]
                sb_w[p] = (load(w1, (32, HID), f32r), load(WLR, (HID, 512), f32r),
                           load(bRow, (1, 512), f32r), load(gb, (HID, 1)))
            sb_Wq = load(Wq_d, (128, 64))
            sb_qb = load(qbRow_d, (1, 64))
            sb_fTq = load(fTq_d, (128, NCP))
            sb_b1 = load(b1_d, (128, T, 8))
            sb_b2 = load(b2_d, (128, T, 8))
            sb_isrc = load(isrc_d, (128, ES // 16), i16)
            sb_idst = load(idst_d, (128, ES // 16), i16)
            sb_iout = load(iout_d, (128, ES // 16), i16)
            sb_m = {k: load(masks_d[k], (128, T4)) for k in masks_d}

            ones = cst.tile([1, 512], f32)
            nc.vector.memset(ones[:], 1.0)
            ones_r = load(ones_d, (1, 512), f32r)
            ident = load(ident_d, (64, 64))

            # ---- init DRAM accumulators
            zt = cst.tile([128, (NCH + 1) * 64], f32)
            nc.vector.memset(zt[:], 0.0)
            et = cst.tile([128, (NCH + 1) * 64], f32)
            nc.vector.memset(et[:], 1e-30)
            flatq = AP(q_dram[:], 0, [[(NCH + 1) * 64, 128], [1, (NCH + 1) * 64]])
            flatwv = AP(wv_dram[:], 0, [[(NCH + 1) * 64, 128], [1, (NCH + 1) * 64]])
            flatden = AP(den_dram[:], 0, [[(NCH + 1) * 64, 128], [1, (NCH + 1) * 64]])
            nc.sync.dma_start(out=flatq, in_=zt[:])
            nc.sync.dma_start(out=flatwv, in_=zt[:])
            nc.sync.dma_start(out=flatden, in_=et[:])

            # ---- Q phase: q[(o*4+d), n] = sum W f ; write q_dram rows
            QCH = 256 if NCP % 256 == 0 else 128
            for c in range(NCP // QCH):
                q_ps = psq.tile([64, QCH], f32, tag="qps")
                nc.tensor.matmul(q_ps[:], sb_Wq[:],
                                 sb_fTq[:, c * QCH:(c + 1) * QCH],
                                 start=True, stop=False)
                nc.tensor.matmul(q_ps[:], sb_qb[:], ones[:, :QCH],
                                 start=False, stop=True)
                q_sb = sm.tile([64, QCH], f32, tag="qsb")
                nc.scalar.copy(q_sb[:], q_ps[:])
                for i in range(QCH // 128):
                    tp = psq.tile([128, 64], f32, tag="qtp")
                    nc.tensor.transpose(tp[:], q_sb[:, i * 128:(i + 1) * 128],
                                        ident[:])
                    qrow = sm.tile([128, 64], f32, tag="qrow")
                    nc.scalar.copy(qrow[:], tp[:])
                    nc.sync.dma_start(
                        out=q_dram[c * QCH + i * 128: c * QCH + (i + 1) * 128, :],
                        in_=qrow[:])

            # ---- persistent edge-major buffers
            vEM = big.tile([128, T, 64], f32)
            scoresF = big.tile([128, T4], f32)   # (head, t) flattened

            # ---- conv main loop: chunks of 8 groups (1024 edges)
            for c in range(CH):
                ef_t = ld.tile([32, 1024], f32r, tag="ef")
                nc.sync.dma_start(out=ef_t[:],
                                  in_=efT_d[:, c * 1024:(c + 1) * 1024])
                fsrc = ld.tile([128, 8, 64], f32, tag="fsrc")
                nc.gpsimd.dma_gather(fsrc[:], fG_d[:],
                                     sb_isrc[:, c * 64:(c + 1) * 64],
                                     1024, 1024, 64)
                qdst = ld.tile([128, 8, 64], f32, tag="qdst")
                nc.gpsimd.dma_gather(qdst[:], q_dram[:],
                                     sb_idst[:, c * 64:(c + 1) * 64],
                                     1024, 1024, 64)
                h_sb = {}
                for p in ("k", "v"):
                    w1, WLR, bRow, gb = sb_w[p]
                    hp = ps.tile([128, 2, 512], f32, tag="hp", bufs=1)
                    for half in range(2):
                        nc.tensor.matmul(
                            hp[:, half, :], w1[:],
                            ef_t[:, half * 512:(half + 1) * 512],
                            start=True, stop=True)
                    h_sb[p] = ld.tile([128, 1024], f32r, tag="h" + p,
                                      name="h" + p)
                    hpf = AP(hp[:], 0, [[1024, 128], [1, 1024]])
                    if GELU_MODE == "hw":
                        nc.scalar.activation(h_sb[p][:], hpf, Act.Gelu,
                                             bias=gb[:], scale=1.0)
                    else:
                        # tanh-approx gelu for CoreSim (no Gelu LUT in sim)
                        xb = ld.tile([128, 1024], f32, tag="xb", name="xb")
                        nc.scalar.activation(xb[:], hpf, Act.Identity,
                                             bias=gb[:], scale=1.0)
                        x2 = ld.tile([128, 1024], f32, tag="x2", name="x2")
                        nc.vector.tensor_mul(x2[:], xb[:], xb[:])
                        x3 = ld.tile([128, 1024], f32, tag="x3", name="x3")
                        nc.vector.scalar_tensor_tensor(
                            x3[:], x2[:], 0.0356774081, xb[:],
                            op0=Alu.mult, op1=Alu.mult)
                        nc.vector.scalar_tensor_tensor(
                            x3[:], xb[:], 0.7978845608, x3[:],
                            op0=Alu.mult, op1=Alu.add)
                        nc.scalar.activation(x3[:], x3[:], Act.Tanh)
                        nc.vector.scalar_tensor_tensor(
                            x3[:], x3[:], 1.0, xb[:],
                            op0=Alu.add, op1=Alu.mult)
                        nc.scalar.activation(h_sb[p][:], x3[:], Act.Copy,
                                             scale=0.5)

                for gg in range(4):  # 2 x 128-edge groups per superblock
                    g0 = gg * 2
                    t0 = c * 8 + g0
                    # tvec = contract(f_src, b1): (m,l) per edge [per group]
                    tvec = ld.tile([128, 2, 32], f32, tag="tvec")
                    for s in range(2):
                        fs = fsrc[:, g0 + s, :]
                        p1 = ld.tile([128, 16, 2, 4], f32, tag="p1")
                        nc.vector.tensor_mul(
                            p1[:],
                            AP(fs, 0, [[512, 128], [4, 16], [0, 2], [1, 4]]),
                            AP(sb_b1[:], (t0 + s) * 8,
                               [[T * 8, 128], [0, 16], [1, 2], [2, 4]]))
                        nc.vector.tensor_reduce(
                            tvec[:, s, :].rearrange("p (m l) -> p m l", l=2),
                            p1[:], axis=AxX, op=Alu.add)
                    kvout = {}
                    for p in ("k", "v"):
                        w1, WLR, bRow, gb = sb_w[p]
                        LR = ps.tile([128, 2, 512], f32, tag="LR")
                        for s in range(2):
                            nc.tensor.matmul(
                                LR[:, s, :],
                                h_sb[p][:, (g0 + s) * 128:(g0 + s + 1) * 128],
                                WLR[:], start=True, stop=False)
                            nc.tensor.matmul(LR[:, s, :], ones_r[:, :128],
                                             bRow[:], start=False, stop=True)
                        # tr[g,r] = sum_c right[g,r,c]*tvec[g,c]
                        p2 = ld.tile([128, 2, 8, 32], f32, tag="p2")
                        nc.vector.tensor_mul(
                            p2[:],
                            AP(LR[:], 256, [[1024, 128], [512, 2], [32, 8], [1, 32]]),
                            AP(tvec[:], 0, [[64, 128], [32, 2], [0, 8], [1, 32]]))
                        tr = ld.tile([128, 2, 8], f32, tag="tr")
                        nc.vector.tensor_reduce(tr[:], p2[:], axis=AxX,
                                                op=Alu.add)
                        # tout[g,c'] = sum_r left[g,c',r]*tr[g,r]
                        p3 = ld.tile([128, 2, 32, 8], f32, tag="p3")
                        nc.vector.tensor_mul(
                            p3[:],
                            AP(LR[:], 0, [[1024, 128], [512, 2], [8, 32], [1, 8]]),
                            AP(tr[:], 0, [[16, 128], [8, 2], [0, 32], [1, 8]]))
                        tout = ld.tile([128, 2, 32], f32, tag="tout")
                        nc.vector.tensor_reduce(tout[:], p3[:], axis=AxX,
                                                op=Alu.add)
                        # out[g,(m,d)] = sum_l tout[g,(m,l)]*b2[g,(l,d)] [per grp]
                        if p == "k":
                            kk = ld.tile([128, 2, 16, 4], f32, tag="kk")
                            kvout[p] = kk
                        for s in range(2):
                            p4 = ld.tile([128, 16, 4, 2], f32, tag="p4")
                            nc.vector.tensor_mul(
                                p4[:],
                                AP(tout[:], s * 32,
                                   [[64, 128], [2, 16], [0, 4], [1, 2]]),
                                AP(sb_b2[:], (t0 + s) * 8,
                                   [[T * 8, 128], [0, 16], [1, 4], [4, 2]]))
                            if p == "k":
                                nc.vector.tensor_reduce(kvout[p][:, s], p4[:],
                                                        axis=AxX, op=Alu.add)
                            else:
                                nc.vector.tensor_reduce(
                                    vEM[:, t0 + s, :].rearrange(
                                        "p (m d) -> p m d", d=4),
                                    p4[:], axis=AxX, op=Alu.add)
                    # scores[g,h] = sum_f qdst[g,h,f]*k[g,h,f] (scale in q)
                    pS = ld.tile([128, 2, 4, 16], f32, tag="pS")
                    nc.vector.tensor_mul(
                        pS[:],
                        AP(qdst[:, g0, :], 0, [[512, 128], [16, 8], [1, 16]]),
                        AP(kvout["k"][:], 0, [[128, 128], [16, 8], [1, 16]]))
                    nc.vector.tensor_reduce(
                        AP(scoresF[:], t0, [[T4, 128], [1, 2], [T, 4]]),
                        pS[:], axis=AxX, op=Alu.add)

            if DEBUG:
                d_vEM = nc.declare_dram_parameter("d_vEM", [128, T * 64], f32, isOutput=True)
                d_sc = nc.declare_dram_parameter("d_sc", [128, T4], f32, isOutput=True)
                nc.sync.dma_start(
                    out=d_vEM[:], in_=AP(vEM[:], 0, [[T * 64, 128], [1, T * 64]]))
                nc.sync.dma_start(out=d_sc[:], in_=scoresF[:])

            # ---- segment softmax over (head, t) rows
            mrun = big.tile([128, T4], f32)
            nc.vector.tensor_tensor_scan(mrun[:], sb_m["sneg"][:], scoresF[:],
                                         NEG, op0=Alu.add, op1=Alu.max)
            em = big.tile([128, T4], f32)
            nc.vector.tensor_mul(em[:], mrun[:], sb_m["esel"][:])
            mmax = big.tile([128, T4], f32)

            def rev(t):
                return AP(t[:], T4 - 1, [[T4, 128], [-1, T4]])

            nc.vector.tensor_tensor_scan(rev(mmax), rev(sb_m["ekeep"]), rev(em),
                                         0.0, op0=Alu.mult, op1=Alu.add)
            ex = big.tile([128, T4], f32)
            nc.vector.tensor_sub(ex[:], scoresF[:], mmax[:])
            nc.scalar.activation(ex[:], ex[:], Act.Exp)
            nc.vector.tensor_mul(ex[:], ex[:], sb_m["pad"][:])
            if DEBUG:
                d_mm = nc.declare_dram_parameter("d_mm", [128, T4], f32, isOutput=True)
                d_ex = nc.declare_dram_parameter("d_ex", [128, T4], f32, isOutput=True)
                d_den = nc.declare_dram_parameter("d_den", [QROWS, 64], f32, isOutput=True)
                d_wv = nc.declare_dram_parameter("d_wv", [QROWS, 64], f32, isOutput=True)
                nc.sync.dma_start(out=d_mm[:], in_=mmax[:])
                nc.sync.dma_start(out=d_ex[:], in_=ex[:])

            # weighted v: vEM[p,t,f] *= ex[p, h(f), t]  (strided broadcast AP)
            nc.vector.tensor_mul(
                vEM[:], vEM[:],
                AP(ex[:], 0, [[T4, 128], [1, T], [T, 4], [0, 16]]))

            # segmented running sums (in place).  Each feature column is an
            # independent 2D strided scan; segment-start mask resets state.
            skeep = sb_m["skeep"]          # (128, 4T): 0 at starts, 1 inside
            sk1 = AP(skeep[:], 0, [[T4, 128], [1, T]])
            for fcol in range(64):
                col = AP(vEM[:], fcol, [[T * 64, 128], [64, T]])
                nc.vector.tensor_tensor_scan(col, sk1, col, 0.0,
                                             op0=Alu.mult, op1=Alu.add)
            for hcol in range(4):
                col = AP(ex[:], hcol * T, [[T4, 128], [1, T]])
                nc.vector.tensor_tensor_scan(col, sk1, col, 0.0,
                                             op0=Alu.mult, op1=Alu.add)
            # ex (scanned) replicated to (t, 64) for the denominator payload
            exr = big.tile([128, T, 64], f32, name="exr")
            nc.vector.tensor_copy(
                exr[:], AP(ex[:], 0, [[T4, 128], [1, T], [T, 4], [0, 16]]))

            # ---- scatter segment totals (unique destination rows),
            # chunked to keep each SWDGE call's descriptor count bounded
            TCs = max(d for d in range(1, 17) if T % d == 0)
            NSC = T // TCs
            for dc in range(NSC):
                isl = sb_iout[:, dc * (128 * TCs // 16):(dc + 1) * (128 * TCs // 16)]
                nc.gpsimd.dma_scatter_add(
                    wv_dram[:], vEM[:, dc * TCs:(dc + 1) * TCs, :], isl,
                    128 * TCs, 128 * TCs, 64)
                nc.gpsimd.dma_scatter_add(
                    den_dram[:], exr[:, dc * TCs:(dc + 1) * TCs, :], isl,
                    128 * TCs, 128 * TCs, 64)

            if DEBUG:
                dq = AP(d_den[:], 0, [[64, QROWS], [1, 64]])
                nc.sync.dma_start(out=d_den[:], in_=den_dram[:])
                nc.sync.dma_start(out=d_wv[:], in_=wv_dram[:])

            # ---- readback node-major, out = wv / den
            nodeap = lambda d: AP(d[:], 0, [[64, 128], [128 * 64, NCH], [1, 64]])
            wv_sb = big.tile([128, NCH, 64], f32)
            den_sb = big.tile([128, NCH, 64], f32)
            nc.sync.dma_start(out=wv_sb[:], in_=nodeap(wv_dram))
            nc.sync.dma_start(out=den_sb[:], in_=nodeap(den_dram))
            nc.vector.reciprocal(den_sb[:], den_sb[:])
            nc.vector.tensor_mul(wv_sb[:], wv_sb[:], den_sb[:])
            nc.sync.dma_start(
                out=AP(out_d[:], 0, [[64, 128], [128 * 64, NCH], [1, 64]]),
                in_=wv_sb[:])

    nc.finalize()
    return nc


# ----------------------------------------------------------------------
# Entry point
# ----------------------------------------------------------------------

def kernel(b1, b2, edge_feats, f, src_idx, dst_idx,
           q_w, q_b,
           k_w1, k_b1, k_wl, k_bl, k_wr, k_br,
           v_w1, v_b1, v_wl, v_bl, v_wr, v_br,
           _run=None):
    kv = {"k": (k_w1, k_b1, k_wl, k_bl, k_wr, k_br),
          "v": (v_w1, v_b1, v_wl, v_bl, v_wr, v_br)}
    in_maps, T = _host_prep(b1, b2, edge_feats, f, src_idx, dst_idx,
                            q_w, q_b, kv)
    nc = build_program(T)

    if _run is None:
        from concourse.bass_utils import run_bass_kernel_spmd
        res = run_bass_kernel_spmd(nc, in_maps, list(range(NCORES)))
        outs = [res.results[i]["out"] for i in range(NCORES)]
    else:
        outs = _run(nc, in_maps)

    full = np.concatenate([np.asarray(o)[:NPC] for o in outs], 0)
    return full.reshape(N, MULT, DIM).astype(np.float32)



# revision 4
# speedup vs baseline: 1.2822x; 1.2822x over previous
"""Trainium2 Bass kernel for nn_EquivariantAttention (GNN message passing).

Strategy
--------
* Host: sort edges by destination node; split the 10000 nodes into 8
  contiguous ranges of 1250 (edge counts are ~E/8 each for the uniform
  graph); each NeuronCore owns one node range and all edges pointing into
  it -> no cross-core communication at all.
* Per core the edges are packed into a (128 partitions x T slots) layout
  such that every destination-node segment lives inside one partition
  (host pads partitions with dummy slots).  Slot (p, t) <-> flat index
  j = t*128 + p, which is exactly the layout dma_gather produces.
* PE (fp32r) does the radial-MLP matmuls:
    h = gelu(w1^T @ efT + b)            (feature-major, 128 x E)
    [left|right] = h_tile^T @ [wl|wr]   (edge-major: stationary = h tile)
  plus bias via rank-1 accumulate matmuls.
* DVE does the per-edge small bilinear contractions (basis contract,
  low-rank apply) as multiply + grouped tensor_reduce with free-dim
  broadcast access patterns.
* Segment softmax: scores are stored (head, t) per partition;
  segment max via masked forward scan + reverse broadcast scan
  (tensor_tensor_scan with negative-stride APs); exp on ACT; the
  segment sums (denominator and weighted V) are done with
  dma_scatter_add (CCE add) into DRAM keyed by destination node.
* Final: node-major readback, out = wv_sum * 1/den, DMA out.

kernel(**inputs) takes the full-problem arrays and returns (10000,16,4).
"""

import os
import sys
import numpy as np

for _p in ("/opt/trn_rl_repo", "/root/.axon_site/_ro/trn_rl_repo"):
    if os.path.isdir(_p) and _p not in sys.path:
        sys.path.insert(0, _p)

# --- static problem config (matches reference.py) ---
N = 10000
E = 160000
MULT = 16
NL = 2
DIM = 4
EDGE_DIM = 32
HID = 128
RANK = 8
H = 4
HM = MULT // H
NCORES = 8
NPC = N // NCORES          # nodes per core = 1250
NCP = 1280                 # padded local node rows (multiple of 128)
NCH = NCP // 128           # node chunks = 10
QROWS = NCP + 128          # + junk/zero chunk (junk row index = NCP)

NEG = -3.0e38
GELU_MODE = "hw"   # "hw": ACT Gelu LUT; "sim": tanh-approx composite
DEBUG = False


# ----------------------------------------------------------------------
# Host-side prep
# ----------------------------------------------------------------------

def _pack_core(dst_sorted_local, T):
    """Greedy-pack node segments (sorted local dst ids) into 128 partitions
    with T slots each.  Returns (slot_of_edge (Ec,), node_slot_info) or None
    if T is too small.  dst_sorted_local: local dst id per edge, sorted."""
    Ec = len(dst_sorted_local)
    # segment boundaries
    nodes, starts = np.unique(dst_sorted_local, return_index=True)
    ends = np.append(starts[1:], Ec)
    p = 0
    fill = 0
    seg_part = np.empty(len(nodes), np.int32)
    seg_off = np.empty(len(nodes), np.int32)
    for s in range(len(nodes)):
        ln = ends[s] - starts[s]
        if ln > T:
            return None
        if fill + ln > T:
            p += 1
            fill = 0
            if p >= 128:
                return None
        seg_part[s] = p
        seg_off[s] = fill
        fill += ln
    # slot index per edge
    slot_p = np.empty(Ec, np.int32)
    slot_t = np.empty(Ec, np.int32)
    for s in range(len(nodes)):
        a, b = starts[s], ends[s]
        slot_p[a:b] = seg_part[s]
        slot_t[a:b] = seg_off[s] + np.arange(b - a)
    return slot_p, slot_t, nodes, starts, ends, seg_part, seg_off


def _prep_core(order_c, dst_c, src_c, b1, b2, ef, n0, T):
    """Build all per-core host arrays for one core. order_c: indices into the
    full edge arrays (already sorted by dst)."""
    Ec = len(order_c)
    dst_local = dst_c - n0
    packed = _pack_core(dst_local, T)
    assert packed is not None
    slot_p, slot_t, nodes, starts, ends, seg_part, seg_off = packed

    ES = 128 * T
    # edge-major slot arrays
    b1EM = np.zeros((128, T, 8), np.float32)
    b2EM = np.zeros((128, T, 8), np.float32)
    efT = np.zeros((32, ES), np.float32)
    isrc = np.full(ES, N, np.int64)        # pad -> zero row of f table
    idst = np.full(ES, NCP, np.int64)      # pad -> zero row of q table
    iout = np.full(ES, NCP, np.int64)      # non-end/pad -> junk row

    flat = slot_t * 128 + slot_p
    b1EM[slot_p, slot_t] = b1[order_c].reshape(Ec, 8)
    b2EM[slot_p, slot_t] = b2[order_c].reshape(Ec, 8)
    efT[:, flat] = ef[order_c].T
    isrc[flat] = src_c
    idst[flat] = dst_local
    # scatter index: ONLY the segment-end slot carries the node id (the
    # running-sum scan puts the segment total there); everything else goes
    # to the junk row, so every real output row is written exactly once
    # (dma_scatter_add races on duplicate destinations on HW).
    end_flat = (seg_off + (ends - starts) - 1) * 128 + seg_part
    iout[end_flat] = nodes

    # masks in (p, t): start of segment / end of segment / pad
    start = np.zeros((128, T), np.float32)
    endf = np.zeros((128, T), np.float32)
    pad = np.zeros((128, T), np.float32)
    pad[slot_p, slot_t] = 1.0
    for s in range(len(nodes)):
        start[seg_part[s], seg_off[s]] = 1.0
        endf[seg_part[s], seg_off[s] + (ends[s] - starts[s]) - 1] = 1.0
    # every pad slot is its own segment start (keeps scans bounded)
    start = np.maximum(start, 1.0 - pad)

    def rep4(m):  # (128,T) -> (128, 4*T) head-major
        return np.tile(m[:, None, :], (1, 4, 1)).reshape(128, 4 * T)

    masks = dict(
        sneg=rep4(start * NEG),          # add-reset mask for max scan
        skeep=rep4(1.0 - start),         # mult-keep mask for sum scan (unused now)
        ekeep=rep4((1.0 - endf) * pad),  # reverse-scan keep (0 at seg end & pads)
        esel=rep4(endf),                 # select segment ends
        pad=rep4(pad),
    )

    def wrap16(ix):
        w = ix.reshape(-1, 16).T.astype(np.int16)   # (16, ES//16)
        return np.ascontiguousarray(np.tile(w, (8, 1)))  # replicate to 128

    return dict(
        b1EM=b1EM.reshape(128, T * 8),
        b2EM=b2EM.reshape(128, T * 8),
        efT=efT,
        isrc=wrap16(isrc),
        idst=wrap16(idst),
        iout=wrap16(iout),
        **{"m_" + k: v for k, v in masks.items()},
    )


def _host_prep(b1, b2, edge_feats, f, src_idx, dst_idx,
               q_w, q_b, kv_params):
    src = np.asarray(src_idx).astype(np.int64)
    dst = np.asarray(dst_idx).astype(np.int64)
    order = np.argsort(dst, kind="stable")
    dst_s = dst[order]

    # per-core edge ranges (node split at multiples of NPC)
    bounds = np.searchsorted(dst_s, np.arange(0, N + 1, NPC))
    core_edges = [order[bounds[i]:bounds[i + 1]] for i in range(NCORES)]

    # find a common T
    T = int(max(2, -(-max(len(c) for c in core_edges) // 128)))
    while True:
        T = -(-T // 8) * 8
        ok = True
        for i in range(NCORES):
            dl = dst_s[bounds[i]:bounds[i + 1]] - i * NPC
            if _pack_core(dl, T) is None:
                ok = False
                break
        if ok:
            break
        T += 8

    # shared (per-core identical) weight arrays
    f64 = np.asarray(f, np.float32).reshape(N, 64)
    fG = np.concatenate([f64, np.zeros((1, 64), np.float32)], 0)  # zero row N

    # q: W = q_w.reshape(NL,MULT,MULT)[IDX]; q[n,o,d] = sum_m W[d,o,m] f[n,m,d]
    IDX = np.array([0, 1, 1, 1])
    Wn = np.asarray(q_w, np.float32).reshape(NL, MULT, MULT)
    Wbig = np.zeros((128, 64), np.float32)
    for d in range(DIM):
        Wd = Wn[IDX[d]]                      # (o, m)
        for m in range(MULT):
            for o in range(MULT):
                Wbig[d * 32 + m, o * 4 + d] = Wd[o, m] * 0.25  # fold 1/sqrt(HM*DIM)
    qbRow = np.zeros((1, 64), np.float32)
    qbRow[0, 0::4] = np.asarray(q_b, np.float32)[:, 0] * 0.25

    shared = dict(Wq=Wbig, qbRow=qbRow, fG=fG,
                  ident=np.eye(64, dtype=np.float32),
                  ones512=np.ones((1, 512), np.float32))
    for p in ("k", "v"):
        w1, bb1, wl, bl, wr, br = kv_params[p]
        shared[p + "w1"] = np.asarray(w1, np.float32)              # (32, 128)
        shared[p + "WLR"] = np.concatenate(
            [np.asarray(wl, np.float32), np.asarray(wr, np.float32)], 1)  # (128,512)
        shared[p + "bRow"] = np.concatenate(
            [np.asarray(bl, np.float32), np.asarray(br, np.float32)])[None, :]
        shared[p + "gb"] = np.asarray(bb1, np.float32)[:, None]    # (128,1)

    in_maps = []
    for i in range(NCORES):
        oc = core_edges[i]
        m = _prep_core(oc, dst[oc], src[oc],
                       np.asarray(b1, np.float32), np.asarray(b2, np.float32),
                       np.asarray(edge_feats, np.float32), i * NPC, T)
        # per-core f slice for q matmul: partition d*32+m, col = local node
        fT = np.zeros((128, NCP), np.float32)
        fl = np.asarray(f, np.float32)[i * NPC:(i + 1) * NPC]  # (NPC, 16, 4)
        for d in range(DIM):
            fT[d * 32:d * 32 + 16, :NPC] = fl[:, :, d].T
        m["fTq"] = fT
        m.update(shared)
        in_maps.append(m)
    return in_maps, T


# ----------------------------------------------------------------------
# Bass program
# ----------------------------------------------------------------------

def build_program(T, reps=1):
    import concourse.bass as bass
    import concourse.tile as tile
    from concourse import bacc as bacc_mod
    from concourse import mybir
    from contextlib import nullcontext

    dt = mybir.dt
    f32, i16 = dt.float32, dt.int16
    f32r = dt.float32r
    Alu = mybir.AluOpType
    Act = mybir.ActivationFunctionType
    AxX = mybir.AxisListType.X
    ES = 128 * T
    CH = T // 8
    T4 = 4 * T

    nc = bacc_mod.Bacc()

    def din(name, shape, dtype=f32):
        return nc.declare_dram_parameter(name, list(shape), dtype, isOutput=False)

    # inputs
    efT_d = din("efT", (32, ES), f32r)
    b1_d = din("b1EM", (128, T * 8))
    b2_d = din("b2EM", (128, T * 8))
    fG_d = din("fG", (N + 1, 64))
    fTq_d = din("fTq", (128, NCP))
    isrc_d = din("isrc", (128, ES // 16), i16)
    idst_d = din("idst", (128, ES // 16), i16)
    iout_d = din("iout", (128, ES // 16), i16)
    Wq_d = din("Wq", (128, 64))
    qbRow_d = din("qbRow", (1, 64))
    wparams = {}
    for p in ("k", "v"):
        wparams[p] = (din(p + "w1", (32, HID), f32r),
                      din(p + "WLR", (HID, 512), f32r),
                      din(p + "bRow", (1, 512), f32r), din(p + "gb", (HID, 1)))
    masks_d = {k: din("m_" + k, (128, T4)) for k in
               ("sneg", "skeep", "ekeep", "esel", "pad")}

    out_d = nc.declare_dram_parameter("out", [NCP, 64], f32, isOutput=True)
    q_dram = nc.dram_tensor("q_dram", [QROWS, 64], f32)
    wv_dram = nc.dram_tensor("wv_dram", [QROWS, 64], f32)
    den_dram = nc.dram_tensor("den_dram", [QROWS, 64], f32)

    ident_d = din("ident", (64, 64))
    ones_d = din("ones512", (1, 512), f32r)

    def AP(t, offset, ap):
        return bass.AP(tensor=t.tensor, offset=t.offset + offset, ap=ap)


    from contextlib import ExitStack
    with tile.TileContext(nc) as tc, ExitStack() as st:
        cst = st.enter_context(tc.tile_pool(name="cst", bufs=1))
        big = st.enter_context(tc.tile_pool(name="big", bufs=1))
        ld = st.enter_context(tc.tile_pool(name="ld", bufs=2))
        sm = st.enter_context(tc.tile_pool(name="sm", bufs=2))
        ps = st.enter_context(tc.tile_pool(name="ps", bufs=2, space="PSUM"))
        psq = st.enter_context(tc.tile_pool(name="psq", bufs=1, space="PSUM"))
        # reps>1: repeat the whole kernel body (all DRAM->SBUF input loads,
        # compute, and the output store) inside a hardware loop so one NEFF
        # execution performs `reps` complete, strictly sequential kernel
        # runs (For_i inserts an all-engine barrier between iterations).
        # Used only for timing; the graded path builds with reps=1.
        with (tc.For_i(0, reps) if reps > 1 else nullcontext()):
            # ---- constants to SBUF
            def load(d, shape, dtype=f32):
                t = cst.tile(list(shape), dtype, name="c_" + d.name)
                nc.sync.dma_start(out=t[:], in_=d[:])
                return t

            sb_w = {}
            for p in ("k", "v"):
                w1, WLR, bRow, gb = wparams[p]
                sb_w[p] = (load(w1, (32, HID), f32r), load(WLR, (HID, 512), f32r),
                           load(bRow, (1, 512), f32r), load(gb, (HID, 1)))
            sb_Wq = load(Wq_d, (128, 64))
            sb_qb = load(qbRow_d, (1, 64))
            sb_fTq = load(fTq_d, (128, NCP))
            sb_b1 = load(b1_d, (128, T, 8))
            sb_b2 = load(b2_d, (128, T, 8))
            sb_isrc = load(isrc_d, (128, ES // 16), i16)
            sb_idst = load(idst_d, (128, ES // 16), i16)
            sb_iout = load(iout_d, (128, ES // 16), i16)
            sb_m = {k: load(masks_d[k], (128, T4)) for k in masks_d}

            ones = cst.tile([1, 512], f32)
            nc.vector.memset(ones[:], 1.0)
            ones_r = load(ones_d, (1, 512), f32r)
            ident = load(ident_d, (64, 64))

            # ---- init DRAM accumulators
            zt = cst.tile([128, (NCH + 1) * 64], f32)
            nc.vector.memset(zt[:], 0.0)
            et = cst.tile([128, (NCH + 1) * 64], f32)
            nc.vector.memset(et[:], 1e-30)
            flatq = AP(q_dram[:], 0, [[(NCH + 1) * 64, 128], [1, (NCH + 1) * 64]])
            flatwv = AP(wv_dram[:], 0, [[(NCH + 1) * 64, 128], [1, (NCH + 1) * 64]])
            flatden = AP(den_dram[:], 0, [[(NCH + 1) * 64, 128], [1, (NCH + 1) * 64]])
            nc.sync.dma_start(out=flatq, in_=zt[:])
            nc.sync.dma_start(out=flatwv, in_=zt[:])
            nc.sync.dma_start(out=flatden, in_=et[:])

            # ---- Q phase: q[(o*4+d), n] = sum W f ; write q_dram rows
            QCH = 256 if NCP % 256 == 0 else 128
            for c in range(NCP // QCH):
                q_ps = psq.tile([64, QCH], f32, tag="qps")
                nc.tensor.matmul(q_ps[:], sb_Wq[:],
                                 sb_fTq[:, c * QCH:(c + 1) * QCH],
                                 start=True, stop=False)
                nc.tensor.matmul(q_ps[:], sb_qb[:], ones[:, :QCH],
                                 start=False, stop=True)
                q_sb = sm.tile([64, QCH], f32, tag="qsb")
                nc.scalar.copy(q_sb[:], q_ps[:])
                for i in range(QCH // 128):
                    tp = psq.tile([128, 64], f32, tag="qtp")
                    nc.tensor.transpose(tp[:], q_sb[:, i * 128:(i + 1) * 128],
                                        ident[:])
                    qrow = sm.tile([128, 64], f32, tag="qrow")
                    nc.scalar.copy(qrow[:], tp[:])
                    nc.sync.dma_start(
                        out=q_dram[c * QCH + i * 128: c * QCH + (i + 1) * 128, :],
                        in_=qrow[:])

            # ---- persistent edge-major buffers
            vEM = big.tile([128, T, 64], f32)
            scoresF = big.tile([128, T4], f32)   # (head, t) flattened

            # ---- conv main loop: chunks of 8 groups (1024 edges)
            for c in range(CH):
                ef_t = ld.tile([32, 1024], f32r, tag="ef")
                nc.sync.dma_start(out=ef_t[:],
                                  in_=efT_d[:, c * 1024:(c + 1) * 1024])
                fsrc = ld.tile([128, 8, 64], f32, tag="fsrc")
                nc.gpsimd.dma_gather(fsrc[:], fG_d[:],
                                     sb_isrc[:, c * 64:(c + 1) * 64],
                                     1024, 1024, 64)
                qdst = ld.tile([128, 8, 64], f32, tag="qdst")
                nc.gpsimd.dma_gather(qdst[:], q_dram[:],
                                     sb_idst[:, c * 64:(c + 1) * 64],
                                     1024, 1024, 64)
                h_sb = {}
                for p in ("k", "v"):
                    w1, WLR, bRow, gb = sb_w[p]
                    hp = ps.tile([128, 2, 512], f32, tag="hp", bufs=1)
                    for half in range(2):
                        nc.tensor.matmul(
                            hp[:, half, :], w1[:],
                            ef_t[:, half * 512:(half + 1) * 512],
                            start=True, stop=True)
                    h_sb[p] = ld.tile([128, 1024], f32r, tag="h" + p,
                                      name="h" + p)
                    hpf = AP(hp[:], 0, [[1024, 128], [1, 1024]])
                    if GELU_MODE == "hw":
                        nc.scalar.activation(h_sb[p][:], hpf, Act.Gelu,
                                             bias=gb[:], scale=1.0)
                    else:
                        # tanh-approx gelu for CoreSim (no Gelu LUT in sim)
                        xb = ld.tile([128, 1024], f32, tag="xb", name="xb")
                        nc.scalar.activation(xb[:], hpf, Act.Identity,
                                             bias=gb[:], scale=1.0)
                        x2 = ld.tile([128, 1024], f32, tag="x2", name="x2")
                        nc.vector.tensor_mul(x2[:], xb[:], xb[:])
                        x3 = ld.tile([128, 1024], f32, tag="x3", name="x3")
                        nc.vector.scalar_tensor_tensor(
                            x3[:], x2[:], 0.0356774081, xb[:],
                            op0=Alu.mult, op1=Alu.mult)
                        nc.vector.scalar_tensor_tensor(
                            x3[:], xb[:], 0.7978845608, x3[:],
                            op0=Alu.mult, op1=Alu.add)
                        nc.scalar.activation(x3[:], x3[:], Act.Tanh)
                        nc.vector.scalar_tensor_tensor(
                            x3[:], x3[:], 1.0, xb[:],
                            op0=Alu.add, op1=Alu.mult)
                        nc.scalar.activation(h_sb[p][:], x3[:], Act.Copy,
                                             scale=0.5)

                for gg in range(4):  # 2 x 128-edge groups per superblock
                    g0 = gg * 2
                    t0 = c * 8 + g0
                    # tvec = contract(f_src, b1): (m,l) per edge [per group]
                    tvec = ld.tile([128, 2, 32], f32, tag="tvec")
                    for s in range(2):
                        fs = fsrc[:, g0 + s, :]
                        p1 = ld.tile([128, 16, 2, 4], f32, tag="p1")
                        nc.vector.tensor_mul(
                            p1[:],
                            AP(fs, 0, [[512, 128], [4, 16], [0, 2], [1, 4]]),
                            AP(sb_b1[:], (t0 + s) * 8,
                               [[T * 8, 128], [0, 16], [1, 2], [2, 4]]))
                        nc.vector.tensor_reduce(
                            tvec[:, s, :].rearrange("p (m l) -> p m l", l=2),
                            p1[:], axis=AxX, op=Alu.add)
                    kvout = {}
                    for p in ("k", "v"):
                        w1, WLR, bRow, gb = sb_w[p]
                        LR = ps.tile([128, 2, 512], f32, tag="LR")
                        for s in range(2):
                            nc.tensor.matmul(
                                LR[:, s, :],
                                h_sb[p][:, (g0 + s) * 128:(g0 + s + 1) * 128],
                                WLR[:], start=True, stop=False)
                            nc.tensor.matmul(LR[:, s, :], ones_r[:, :128],
                                             bRow[:], start=False, stop=True)
                        # tr[g,r] = sum_c right[g,r,c]*tvec[g,c]
                        p2 = ld.tile([128, 2, 8, 32], f32, tag="p2")
                        nc.vector.tensor_mul(
                            p2[:],
                            AP(LR[:], 256, [[1024, 128], [512, 2], [32, 8], [1, 32]]),
                            AP(tvec[:], 0, [[64, 128], [32, 2], [0, 8], [1, 32]]))
                        tr = ld.tile([128, 2, 8], f32, tag="tr")
                        nc.vector.tensor_reduce(tr[:], p2[:], axis=AxX,
                                                op=Alu.add)
                        # tout[g,c'] = sum_r left[g,c',r]*tr[g,r]
                        p3 = ld.tile([128, 2, 32, 8], f32, tag="p3")
                        nc.vector.tensor_mul(
                            p3[:],
                            AP(LR[:], 0, [[1024, 128], [512, 2], [8, 32], [1, 8]]),
                            AP(tr[:], 0, [[16, 128], [8, 2], [0, 32], [1, 8]]))
                        tout = ld.tile([128, 2, 32], f32, tag="tout")
                        nc.vector.tensor_reduce(tout[:], p3[:], axis=AxX,
                                                op=Alu.add)
                        # out[g,(m,d)] = sum_l tout[g,(m,l)]*b2[g,(l,d)] [per grp]
                        if p == "k":
                            kk = ld.tile([128, 2, 16, 4], f32, tag="kk")
                            kvout[p] = kk
                        for s in range(2):
                            p4 = ld.tile([128, 16, 4, 2], f32, tag="p4")
                            nc.vector.tensor_mul(
                                p4[:],
                                AP(tout[:], s * 32,
                                   [[64, 128], [2, 16], [0, 4], [1, 2]]),
                                AP(sb_b2[:], (t0 + s) * 8,
                                   [[T * 8, 128], [0, 16], [1, 4], [4, 2]]))
                            if p == "k":
                                nc.vector.tensor_reduce(kvout[p][:, s], p4[:],
                                                        axis=AxX, op=Alu.add)
                            else:
                                nc.vector.tensor_reduce(
                                    vEM[:, t0 + s, :].rearrange(
                                        "p (m d) -> p m d", d=4),
                                    p4[:], axis=AxX, op=Alu.add)
                    # scores[g,h] = sum_f qdst[g,h,f]*k[g,h,f] (scale in q)
                    pS = ld.tile([128, 2, 4, 16], f32, tag="pS")
                    nc.vector.tensor_mul(
                        pS[:],
                        AP(qdst[:, g0, :], 0, [[512, 128], [16, 8], [1, 16]]),
                        AP(kvout["k"][:], 0, [[128, 128], [16, 8], [1, 16]]))
                    nc.vector.tensor_reduce(
                        AP(scoresF[:], t0, [[T4, 128], [1, 2], [T, 4]]),
                        pS[:], axis=AxX, op=Alu.add)

            if DEBUG:
                d_vEM = nc.declare_dram_parameter("d_vEM", [128, T * 64], f32, isOutput=True)
                d_sc = nc.declare_dram_parameter("d_sc", [128, T4], f32, isOutput=True)
                nc.sync.dma_start(
                    out=d_vEM[:], in_=AP(vEM[:], 0, [[T * 64, 128], [1, T * 64]]))
                nc.sync.dma_start(out=d_sc[:], in_=scoresF[:])

            # ---- segment softmax over (head, t) rows
            mrun = big.tile([128, T4], f32)
            nc.vector.tensor_tensor_scan(mrun[:], sb_m["sneg"][:], scoresF[:],
                                         NEG, op0=Alu.add, op1=Alu.max)
            em = big.tile([128, T4], f32)
            nc.vector.tensor_mul(em[:], mrun[:], sb_m["esel"][:])
            mmax = big.tile([128, T4], f32)

            def rev(t):
                return AP(t[:], T4 - 1, [[T4, 128], [-1, T4]])

            nc.vector.tensor_tensor_scan(rev(mmax), rev(sb_m["ekeep"]), rev(em),
                                         0.0, op0=Alu.mult, op1=Alu.add)
            ex = big.tile([128, T4], f32)
            nc.vector.tensor_sub(ex[:], scoresF[:], mmax[:])
            nc.scalar.activation(ex[:], ex[:], Act.Exp)
            nc.vector.tensor_mul(ex[:], ex[:], sb_m["pad"][:])
            if DEBUG:
                d_mm = nc.declare_dram_parameter("d_mm", [128, T4], f32, isOutput=True)
                d_ex = nc.declare_dram_parameter("d_ex", [128, T4], f32, isOutput=True)
                d_den = nc.declare_dram_parameter("d_den", [QROWS, 64], f32, isOutput=True)
                d_wv = nc.declare_dram_parameter("d_wv", [QROWS, 64], f32, isOutput=True)
                nc.sync.dma_start(out=d_mm[:], in_=mmax[:])
                nc.sync.dma_start(out=d_ex[:], in_=ex[:])

            # weighted v: vEM[p,t,f] *= ex[p, h(f), t]  (strided broadcast AP)
            nc.vector.tensor_mul(
                vEM[:], vEM[:],
                AP(ex[:], 0, [[T4, 128], [1, T], [T, 4], [0, 16]]))

            # segmented running sums (in place).  Each feature column is an
            # independent 2D strided scan; segment-start mask resets state.
            skeep = sb_m["skeep"]          # (128, 4T): 0 at starts, 1 inside
            sk1 = AP(skeep[:], 0, [[T4, 128], [1, T]])
            for fcol in range(64):
                col = AP(vEM[:], fcol, [[T * 64, 128], [64, T]])
                nc.vector.tensor_tensor_scan(col, sk1, col, 0.0,
                                             op0=Alu.mult, op1=Alu.add)
            for hcol in range(4):
                col = AP(ex[:], hcol * T, [[T4, 128], [1, T]])
                nc.vector.tensor_tensor_scan(col, sk1, col, 0.0,
                                             op0=Alu.mult, op1=Alu.add)
            # ex (scanned) replicated to (t, 64) for the denominator payload
            exr = big.tile([128, T, 64], f32, name="exr")
            nc.vector.tensor_copy(
                exr[:], AP(ex[:], 0, [[T4, 128], [1, T], [T, 4], [0, 16]]))

            # ---- scatter segment totals (unique destination rows),
            # chunked to keep each SWDGE call's descriptor count bounded
            TCs = max(d for d in range(1, 17) if T % d == 0)
            NSC = T // TCs
            for dc in range(NSC):
                isl = sb_iout[:, dc * (128 * TCs // 16):(dc + 1) * (128 * TCs // 16)]
                nc.gpsimd.dma_scatter_add(
                    wv_dram[:], vEM[:, dc * TCs:(dc + 1) * TCs, :], isl,
                    128 * TCs, 128 * TCs, 64)
                nc.gpsimd.dma_scatter_add(
                    den_dram[:], exr[:, dc * TCs:(dc + 1) * TCs, :], isl,
                    128 * TCs, 128 * TCs, 64)

            if DEBUG:
                dq = AP(d_den[:], 0, [[64, QROWS], [1, 64]])
                nc.sync.dma_start(out=d_den[:], in_=den_dram[:])
                nc.sync.dma_start(out=d_wv[:], in_=wv_dram[:])

            # ---- readback node-major, out = wv / den
            nodeap = lambda d: AP(d[:], 0, [[64, 128], [128 * 64, NCH], [1, 64]])
            wv_sb = big.tile([128, NCH, 64], f32)
            den_sb = big.tile([128, NCH, 64], f32)
            nc.sync.dma_start(out=wv_sb[:], in_=nodeap(wv_dram))
            nc.sync.dma_start(out=den_sb[:], in_=nodeap(den_dram))
            nc.vector.reciprocal(den_sb[:], den_sb[:])
            nc.vector.tensor_mul(wv_sb[:], wv_sb[:], den_sb[:])
            nc.sync.dma_start(
                out=AP(out_d[:], 0, [[64, 128], [128 * 64, NCH], [1, 64]]),
                in_=wv_sb[:])

    nc.finalize()
    return nc


# ----------------------------------------------------------------------
# Entry point
# ----------------------------------------------------------------------

def kernel(b1, b2, edge_feats, f, src_idx, dst_idx,
           q_w, q_b,
           k_w1, k_b1, k_wl, k_bl, k_wr, k_br,
           v_w1, v_b1, v_wl, v_bl, v_wr, v_br,
           _run=None, _reps=1):
    kv = {"k": (k_w1, k_b1, k_wl, k_bl, k_wr, k_br),
          "v": (v_w1, v_b1, v_wl, v_bl, v_wr, v_br)}
    in_maps, T = _host_prep(b1, b2, edge_feats, f, src_idx, dst_idx,
                            q_w, q_b, kv)
    nc = build_program(T, reps=_reps)

    if _run is None:
        from concourse.bass_utils import run_bass_kernel_spmd
        res = run_bass_kernel_spmd(nc, in_maps, list(range(NCORES)))
        outs = [res.results[i]["out"] for i in range(NCORES)]
    else:
        outs = _run(nc, in_maps)

    full = np.concatenate([np.asarray(o)[:NPC] for o in outs], 0)
    return full.reshape(N, MULT, DIM).astype(np.float32)



# revision 16
# speedup vs baseline: 5.2128x; 4.0656x over previous
"""Trainium2 Bass kernel for nn_EquivariantAttention (GNN message passing).

Strategy
--------
* Host: sort edges by destination node; split the 10000 nodes into 8
  contiguous ranges of 1250 (edge counts are ~E/8 each for the uniform
  graph); each NeuronCore owns one node range and all edges pointing into
  it -> no cross-core communication at all.
* Per core the edges are packed into a (128 partitions x T slots) layout
  such that every destination-node segment lives inside one partition
  (host pads partitions with dummy slots).  Slot (p, t) <-> flat index
  j = t*128 + p, which is exactly the layout dma_gather produces.
* PE (fp32r) does the radial-MLP matmuls:
    h = gelu(w1^T @ efT + b)            (feature-major, 128 x E)
    [left|right] = h_tile^T @ [wl|wr]   (edge-major: stationary = h tile)
  plus bias via rank-1 accumulate matmuls.
* DVE does the per-edge small bilinear contractions (basis contract,
  low-rank apply) as multiply + grouped tensor_reduce with free-dim
  broadcast access patterns.
* Segment softmax: scores are stored (head, t) per partition;
  segment max via masked forward scan + reverse broadcast scan
  (tensor_tensor_scan with negative-stride APs); exp on ACT; the
  segment sums (denominator and weighted V) are done with
  dma_scatter_add (CCE add) into DRAM keyed by destination node.
* Final: node-major readback, out = wv_sum * 1/den, DMA out.

kernel(**inputs) takes the full-problem arrays and returns (10000,16,4).
"""

import os
import sys
import numpy as np

for _p in ("/opt/trn_rl_repo", "/root/.axon_site/_ro/trn_rl_repo"):
    if os.path.isdir(_p) and _p not in sys.path:
        sys.path.insert(0, _p)

# --- static problem config (matches reference.py) ---
N = 10000
E = 160000
MULT = 16
NL = 2
DIM = 4
EDGE_DIM = 32
HID = 128
RANK = 8
H = 4
HM = MULT // H
NCORES = 8
NPC = N // NCORES          # nodes per core = 1250
NCP = 1280                 # padded local node rows (multiple of 128)
NCH = NCP // 128           # node chunks = 10
QROWS = NCP + 128          # + junk/zero chunk (junk row index = NCP)

NEG = -3.0e38
GELU_MODE = "hw"   # "hw": ACT Gelu LUT; "sim": tanh-approx composite
DEBUG = False
ABLATE = os.environ.get("ABLATE", "")   # timing probes: "g"=no gathers, "s"=no scatters


# ----------------------------------------------------------------------
# Host-side prep
# ----------------------------------------------------------------------

def _pack_core(dst_sorted_local, T):
    """Greedy-pack node segments (sorted local dst ids) into 128 partitions
    with T slots each.  Returns (slot_of_edge (Ec,), node_slot_info) or None
    if T is too small.  dst_sorted_local: local dst id per edge, sorted."""
    Ec = len(dst_sorted_local)
    # segment boundaries
    nodes, starts = np.unique(dst_sorted_local, return_index=True)
    ends = np.append(starts[1:], Ec)
    p = 0
    fill = 0
    seg_part = np.empty(len(nodes), np.int32)
    seg_off = np.empty(len(nodes), np.int32)
    for s in range(len(nodes)):
        ln = ends[s] - starts[s]
        if ln > T:
            return None
        if fill + ln > T:
            p += 1
            fill = 0
            if p >= 128:
                return None
        seg_part[s] = p
        seg_off[s] = fill
        fill += ln
    # slot index per edge
    slot_p = np.empty(Ec, np.int32)
    slot_t = np.empty(Ec, np.int32)
    for s in range(len(nodes)):
        a, b = starts[s], ends[s]
        slot_p[a:b] = seg_part[s]
        slot_t[a:b] = seg_off[s] + np.arange(b - a)
    return slot_p, slot_t, nodes, starts, ends, seg_part, seg_off


def _prep_core(order_c, dst_c, src_c, b1, b2, ef, n0, T):
    """Build all per-core host arrays for one core. order_c: indices into the
    full edge arrays (already sorted by dst)."""
    Ec = len(order_c)
    dst_local = dst_c - n0
    packed = _pack_core(dst_local, T)
    assert packed is not None
    slot_p, slot_t, nodes, starts, ends, seg_part, seg_off = packed

    ES = 128 * T
    # edge-major slot arrays
    b1EM = np.zeros((128, T, 8), np.float32)
    b2EM = np.zeros((128, T, 8), np.float32)
    efT = np.zeros((32, ES), np.float32)
    isrc = np.full(ES, N, np.int64)        # pad -> zero row of f table
    idst = np.full(ES, NCP, np.int64)      # pad -> zero row of q table

    flat = slot_t * 128 + slot_p
    b1EM[slot_p, slot_t] = b1[order_c].reshape(Ec, 8)
    b2EM[slot_p, slot_t] = b2[order_c].reshape(Ec, 8)
    efT[:, flat] = ef[order_c].T
    isrc[flat] = src_c
    idst[flat] = dst_local
    # node-major output gather index: for local node n, the flat slot id of
    # its segment-END slot (the running-sum scan puts the segment total
    # there).  Nodes with no incoming edges (and the NCP padding rows) point
    # at a pad slot, whose scanned wv and ex are both 0 -> after the den
    # eps-floor the gathered output row is exactly 0, matching the
    # reference.
    end_flat = (seg_off + (ends - starts) - 1) * 128 + seg_part
    pad_free = np.ones((128, T), bool)
    pad_free[slot_p, slot_t] = False
    pf = np.argwhere(pad_free)
    if len(pf):
        default_slot = pf[0][1] * 128 + pf[0][0]
    else:
        assert len(nodes) == NPC, "no pad slot available for edgeless nodes"
        default_slot = 0
    iend = np.full(NCP, default_slot, np.int64)
    iend[nodes] = end_flat

    # masks in (p, t): start of segment / end of segment / pad
    start = np.zeros((128, T), np.float32)
    endf = np.zeros((128, T), np.float32)
    pad = np.zeros((128, T), np.float32)
    pad[slot_p, slot_t] = 1.0
    for s in range(len(nodes)):
        start[seg_part[s], seg_off[s]] = 1.0
        endf[seg_part[s], seg_off[s] + (ends[s] - starts[s]) - 1] = 1.0
    # every pad slot is its own segment start (keeps scans bounded)
    start = np.maximum(start, 1.0 - pad)

    def rep4(m):  # (128,T) -> (128, 4*T) head-major
        return np.tile(m[:, None, :], (1, 4, 1)).reshape(128, 4 * T)

    masks = dict(
        sneg=rep4(start * NEG),          # add-reset mask for max scan
        skeep=rep4(1.0 - start),         # mult-keep mask for sum scan (unused now)
        ekeep=rep4((1.0 - endf) * pad),  # reverse-scan keep (0 at seg end & pads)
        esel=rep4(endf),                 # select segment ends
        pad=rep4(pad),
    )

    def wrap16(ix):
        w = ix.reshape(-1, 16).T.astype(np.int16)   # (16, len//16)
        return np.ascontiguousarray(np.tile(w, (8, 1)))  # replicate to 128

    return dict(
        b1EM=b1EM.reshape(128, T * 8),
        b2EM=b2EM.reshape(128, T * 8),
        efT=efT,
        isrc=wrap16(isrc),
        idst=wrap16(idst),
        iend=wrap16(iend),
        **{"m_" + k: v for k, v in masks.items()},
    )


def _host_prep(b1, b2, edge_feats, f, src_idx, dst_idx,
               q_w, q_b, kv_params):
    src = np.asarray(src_idx).astype(np.int64)
    dst = np.asarray(dst_idx).astype(np.int64)
    order = np.argsort(dst, kind="stable")
    dst_s = dst[order]

    # per-core edge ranges (node split at multiples of NPC)
    bounds = np.searchsorted(dst_s, np.arange(0, N + 1, NPC))
    core_edges = [order[bounds[i]:bounds[i + 1]] for i in range(NCORES)]

    # find a common T
    T = int(max(2, -(-max(len(c) for c in core_edges) // 128)))
    while True:
        T = -(-T // 8) * 8
        ok = True
        for i in range(NCORES):
            dl = dst_s[bounds[i]:bounds[i + 1]] - i * NPC
            if _pack_core(dl, T) is None:
                ok = False
                break
        if ok:
            break
        T += 8

    # shared (per-core identical) weight arrays
    f64 = np.asarray(f, np.float32).reshape(N, 64)
    fG = np.concatenate([f64, np.zeros((1, 64), np.float32)], 0)  # zero row N

    # q: W = q_w.reshape(NL,MULT,MULT)[IDX]; q[n,o,d] = sum_m W[d,o,m] f[n,m,d]
    IDX = np.array([0, 1, 1, 1])
    Wn = np.asarray(q_w, np.float32).reshape(NL, MULT, MULT)
    Wbig = np.zeros((128, 64), np.float32)
    for d in range(DIM):
        Wd = Wn[IDX[d]]                      # (o, m)
        for m in range(MULT):
            for o in range(MULT):
                Wbig[d * 32 + m, o * 4 + d] = Wd[o, m] * 0.25  # fold 1/sqrt(HM*DIM)
    qbRow = np.zeros((1, 64), np.float32)
    qbRow[0, 0::4] = np.asarray(q_b, np.float32)[:, 0] * 0.25

    shared = dict(Wq=Wbig, qbRow=qbRow, fG=fG,
                  ident=np.eye(64, dtype=np.float32),
                  ones512=np.ones((1, 512), np.float32))
    for p in ("k", "v"):
        w1, bb1, wl, bl, wr, br = kv_params[p]
        shared[p + "w1"] = np.asarray(w1, np.float32)              # (32, 128)
        shared[p + "WLR"] = np.concatenate(
            [np.asarray(wl, np.float32), np.asarray(wr, np.float32)], 1)  # (128,512)
        shared[p + "bRow"] = np.concatenate(
            [np.asarray(bl, np.float32), np.asarray(br, np.float32)])[None, :]
        shared[p + "gb"] = np.asarray(bb1, np.float32)[:, None]    # (128,1)

    in_maps = []
    for i in range(NCORES):
        oc = core_edges[i]
        m = _prep_core(oc, dst[oc], src[oc],
                       np.asarray(b1, np.float32), np.asarray(b2, np.float32),
                       np.asarray(edge_feats, np.float32), i * NPC, T)
        # per-core f slice for q matmul: partition d*32+m, col = local node
        fT = np.zeros((128, NCP), np.float32)
        fl = np.asarray(f, np.float32)[i * NPC:(i + 1) * NPC]  # (NPC, 16, 4)
        for d in range(DIM):
            fT[d * 32:d * 32 + 16, :NPC] = fl[:, :, d].T
        m["fTq"] = fT
        m.update(shared)
        in_maps.append(m)
    return in_maps, T


# ----------------------------------------------------------------------
# Bass program
# ----------------------------------------------------------------------

def build_program(T, reps=1):
    import concourse.bass as bass
    import concourse.tile as tile
    from concourse import bacc as bacc_mod
    from concourse import mybir
    from contextlib import nullcontext

    dt = mybir.dt
    f32, i16 = dt.float32, dt.int16
    f32r = dt.float32r
    Alu = mybir.AluOpType
    Act = mybir.ActivationFunctionType
    AxX = mybir.AxisListType.X
    ES = 128 * T
    CH = T // 8
    T4 = 4 * T

    nc = bacc_mod.Bacc(num_swdge_queues=4)

    def din(name, shape, dtype=f32):
        return nc.declare_dram_parameter(name, list(shape), dtype, isOutput=False)

    # inputs
    efT_d = din("efT", (32, ES), f32r)
    b1_d = din("b1EM", (128, T * 8))
    b2_d = din("b2EM", (128, T * 8))
    fG_d = din("fG", (N + 1, 64))
    fTq_d = din("fTq", (128, NCP))
    isrc_d = din("isrc", (128, ES // 16), i16)
    idst_d = din("idst", (128, ES // 16), i16)
    iend_d = din("iend", (128, NCP // 16), i16)
    Wq_d = din("Wq", (128, 64))
    qbRow_d = din("qbRow", (1, 64))
    wparams = {}
    for p in ("k", "v"):
        wparams[p] = (din(p + "w1", (32, HID), f32r),
                      din(p + "WLR", (HID, 512), f32r),
                      din(p + "bRow", (1, 512), f32r), din(p + "gb", (HID, 1)))
    masks_d = {k: din("m_" + k, (128, T4)) for k in
               ("sneg", "skeep", "ekeep", "esel", "pad")}

    out_d = nc.declare_dram_parameter("out", [NCP, 64], f32, isOutput=True)
    q_dram = nc.dram_tensor("q_dram", [QROWS, 64], f32)
    vd_dram = nc.dram_tensor("vd_dram", [ES, 128], f32)

    ident_d = din("ident", (64, 64))
    ones_d = din("ones512", (1, 512), f32r)

    def AP(t, offset, ap):
        return bass.AP(tensor=t.tensor, offset=t.offset + offset, ap=ap)


    from contextlib import ExitStack
    with tile.TileContext(nc) as tc, ExitStack() as st:
        cst = st.enter_context(tc.tile_pool(name="cst", bufs=1))
        big = st.enter_context(tc.tile_pool(name="big", bufs=1))
        ld = st.enter_context(tc.tile_pool(name="ld", bufs=2))
        sm = st.enter_context(tc.tile_pool(name="sm", bufs=2))
        ps = st.enter_context(tc.tile_pool(name="ps", bufs=2, space="PSUM"))
        psq = st.enter_context(tc.tile_pool(name="psq", bufs=1, space="PSUM"))
        # reps>1: repeat the whole kernel body (all DRAM->SBUF input loads,
        # compute, and the output store) inside a hardware loop so one NEFF
        # execution performs `reps` complete, strictly sequential kernel
        # runs (For_i inserts an all-engine barrier between iterations).
        # Used only for timing; the graded path builds with reps=1.
        with (tc.For_i(0, reps) if reps > 1 else nullcontext()):
            # ---- constants to SBUF
            def load(d, shape, dtype=f32):
                t = cst.tile(list(shape), dtype, name="c_" + d.name)
                nc.sync.dma_start(out=t[:], in_=d[:])
                return t

            sb_w = {}
            for p in ("k", "v"):
                w1, WLR, bRow, gb = wparams[p]
                sb_w[p] = (load(w1, (32, HID), f32r), load(WLR, (HID, 512), f32r),
                           load(bRow, (1, 512), f32r), load(gb, (HID, 1)))
            sb_Wq = load(Wq_d, (128, 64))
            sb_qb = load(qbRow_d, (1, 64))
            sb_fTq = load(fTq_d, (128, NCP))
            sb_b1 = load(b1_d, (128, T, 8))
            sb_b2 = load(b2_d, (128, T, 8))
            sb_isrc = load(isrc_d, (128, ES // 16), i16)
            sb_idst = load(idst_d, (128, ES // 16), i16)
            sb_iend = load(iend_d, (128, NCP // 16), i16)
            sb_m = {k: load(masks_d[k], (128, T4)) for k in masks_d}

            ones = cst.tile([1, 512], f32)
            nc.vector.memset(ones[:], 1.0)
            ones_r = load(ones_d, (1, 512), f32r)
            ident = load(ident_d, (64, 64))

            # zero the junk q rows (qdst pad-gather target) so pad scores
            # stay finite (NaN would poison the segment-max scan)
            zq = cst.tile([128, 64], f32)
            nc.vector.memset(zq[:], 0.0)
            nc.sync.dma_start(out=q_dram[NCP:QROWS, :], in_=zq[:])

            # ---- Q phase: q[(o*4+d), n] = sum W f ; write q_dram rows
            QCH = 256 if NCP % 256 == 0 else 128
            for c in range(NCP // QCH):
                q_ps = psq.tile([64, QCH], f32, tag="qps")
                nc.tensor.matmul(q_ps[:], sb_Wq[:],
                                 sb_fTq[:, c * QCH:(c + 1) * QCH],
                                 start=True, stop=False)
                nc.tensor.matmul(q_ps[:], sb_qb[:], ones[:, :QCH],
                                 start=False, stop=True)
                q_sb = sm.tile([64, QCH], f32, tag="qsb")
                nc.scalar.copy(q_sb[:], q_ps[:])
                for i in range(QCH // 128):
                    tp = psq.tile([128, 64], f32, tag="qtp")
                    nc.tensor.transpose(tp[:], q_sb[:, i * 128:(i + 1) * 128],
                                        ident[:])
                    qrow = sm.tile([128, 64], f32, tag="qrow")
                    nc.scalar.copy(qrow[:], tp[:])
                    nc.sync.dma_start(
                        out=q_dram[c * QCH + i * 128: c * QCH + (i + 1) * 128, :],
                        in_=qrow[:])

            # ---- persistent edge-major buffers
            vEM = big.tile([128, T, 64], f32)
            scoresF = big.tile([128, T4], f32)   # (head, t) flattened

            # ---- conv main loop: chunks of 8 groups (1024 edges)
            for c in range(CH):
                ef_t = ld.tile([32, 1024], f32r, tag="ef")
                nc.sync.dma_start(out=ef_t[:],
                                  in_=efT_d[:, c * 1024:(c + 1) * 1024])
                fsrc = ld.tile([128, 8, 64], f32, tag="fsrc")
                qdst = ld.tile([128, 8, 64], f32, tag="qdst")
                if "g" in ABLATE:
                    nc.sync.dma_start(out=fsrc[:], in_=AP(
                        fG_d[:], 0, [[64, 128], [128 * 64, 8], [1, 64]]))
                    nc.sync.dma_start(out=qdst[:], in_=AP(
                        q_dram[:], 0, [[64, 128], [128 * 64, 8], [1, 64]]))
                else:
                    nc.gpsimd.dma_gather(fsrc[:], fG_d[:],
                                         sb_isrc[:, c * 64:(c + 1) * 64],
                                         1024, 1024, 64,
                                         queue_num=(2 * c) % 4)
                    nc.gpsimd.dma_gather(qdst[:], q_dram[:],
                                         sb_idst[:, c * 64:(c + 1) * 64],
                                         1024, 1024, 64,
                                         queue_num=(2 * c + 1) % 4)
                h_sb = {}
                for p in ("k", "v"):
                    w1, WLR, bRow, gb = sb_w[p]
                    hp = ps.tile([128, 2, 512], f32, tag="hp", bufs=1)
                    for half in range(2):
                        nc.tensor.matmul(
                            hp[:, half, :], w1[:],
                            ef_t[:, half * 512:(half + 1) * 512],
                            start=True, stop=True)
                    h_sb[p] = ld.tile([128, 1024], f32r, tag="h" + p,
                                      name="h" + p)
                    hpf = AP(hp[:], 0, [[1024, 128], [1, 1024]])
                    if GELU_MODE == "hw":
                        nc.scalar.activation(h_sb[p][:], hpf, Act.Gelu,
                                             bias=gb[:], scale=1.0)
                    else:
                        # tanh-approx gelu for CoreSim (no Gelu LUT in sim)
                        xb = ld.tile([128, 1024], f32, tag="xb", name="xb")
                        nc.scalar.activation(xb[:], hpf, Act.Identity,
                                             bias=gb[:], scale=1.0)
                        x2 = ld.tile([128, 1024], f32, tag="x2", name="x2")
                        nc.vector.tensor_mul(x2[:], xb[:], xb[:])
                        x3 = ld.tile([128, 1024], f32, tag="x3", name="x3")
                        nc.vector.scalar_tensor_tensor(
                            x3[:], x2[:], 0.0356774081, xb[:],
                            op0=Alu.mult, op1=Alu.mult)
                        nc.vector.scalar_tensor_tensor(
                            x3[:], xb[:], 0.7978845608, x3[:],
                            op0=Alu.mult, op1=Alu.add)
                        nc.scalar.activation(x3[:], x3[:], Act.Tanh)
                        nc.vector.scalar_tensor_tensor(
                            x3[:], x3[:], 1.0, xb[:],
                            op0=Alu.add, op1=Alu.mult)
                        nc.scalar.activation(h_sb[p][:], x3[:], Act.Copy,
                                             scale=0.5)

                for gg in range(4):  # 2 x 128-edge groups per superblock
                    g0 = gg * 2
                    t0 = c * 8 + g0
                    # tvec = contract(f_src, b1): (m,l) per edge [per group]
                    tvec = ld.tile([128, 2, 32], f32, tag="tvec")
                    for s in range(2):
                        fs = fsrc[:, g0 + s, :]
                        p1 = ld.tile([128, 16, 2, 4], f32, tag="p1")
                        nc.vector.tensor_mul(
                            p1[:],
                            AP(fs, 0, [[512, 128], [4, 16], [0, 2], [1, 4]]),
                            AP(sb_b1[:], (t0 + s) * 8,
                               [[T * 8, 128], [0, 16], [1, 2], [2, 4]]))
                        nc.vector.tensor_reduce(
                            tvec[:, s, :].rearrange("p (m l) -> p m l", l=2),
                            p1[:], axis=AxX, op=Alu.add)
                    kvout = {}
                    for p in ("k", "v"):
                        w1, WLR, bRow, gb = sb_w[p]
                        LR = ps.tile([128, 2, 512], f32, tag="LR")
                        for s in range(2):
                            nc.tensor.matmul(
                                LR[:, s, :],
                                h_sb[p][:, (g0 + s) * 128:(g0 + s + 1) * 128],
                                WLR[:], start=True, stop=False)
                            nc.tensor.matmul(LR[:, s, :], ones_r[:, :128],
                                             bRow[:], start=False, stop=True)
                        # tr[g,r] = sum_c right[g,r,c]*tvec[g,c]
                        p2 = ld.tile([128, 2, 8, 32], f32, tag="p2")
                        nc.vector.tensor_mul(
                            p2[:],
                            AP(LR[:], 256, [[1024, 128], [512, 2], [32, 8], [1, 32]]),
                            AP(tvec[:], 0, [[64, 128], [32, 2], [0, 8], [1, 32]]))
                        tr = ld.tile([128, 2, 8], f32, tag="tr")
                        nc.vector.tensor_reduce(tr[:], p2[:], axis=AxX,
                                                op=Alu.add)
                        # tout[g,c'] = sum_r left[g,c',r]*tr[g,r]
                        p3 = ld.tile([128, 2, 32, 8], f32, tag="p3")
                        nc.vector.tensor_mul(
                            p3[:],
                            AP(LR[:], 0, [[1024, 128], [512, 2], [8, 32], [1, 8]]),
                            AP(tr[:], 0, [[16, 128], [8, 2], [0, 32], [1, 8]]))
                        tout = ld.tile([128, 2, 32], f32, tag="tout")
                        nc.vector.tensor_reduce(tout[:], p3[:], axis=AxX,
                                                op=Alu.add)
                        # out[g,(m,d)] = sum_l tout[g,(m,l)]*b2[g,(l,d)] [per grp]
                        if p == "k":
                            kk = ld.tile([128, 2, 16, 4], f32, tag="kk")
                            kvout[p] = kk
                        for s in range(2):
                            p4 = ld.tile([128, 16, 4, 2], f32, tag="p4")
                            nc.vector.tensor_mul(
                                p4[:],
                                AP(tout[:], s * 32,
                                   [[64, 128], [2, 16], [0, 4], [1, 2]]),
                                AP(sb_b2[:], (t0 + s) * 8,
                                   [[T * 8, 128], [0, 16], [1, 4], [4, 2]]))
                            if p == "k":
                                nc.vector.tensor_reduce(kvout[p][:, s], p4[:],
                                                        axis=AxX, op=Alu.add)
                            else:
                                nc.vector.tensor_reduce(
                                    vEM[:, t0 + s, :].rearrange(
                                        "p (m d) -> p m d", d=4),
                                    p4[:], axis=AxX, op=Alu.add)
                    # scores[g,h] = sum_f qdst[g,h,f]*k[g,h,f] (scale in q)
                    pS = ld.tile([128, 2, 4, 16], f32, tag="pS")
                    nc.vector.tensor_mul(
                        pS[:],
                        AP(qdst[:, g0, :], 0, [[512, 128], [16, 8], [1, 16]]),
                        AP(kvout["k"][:], 0, [[128, 128], [16, 8], [1, 16]]))
                    nc.vector.tensor_reduce(
                        AP(scoresF[:], t0, [[T4, 128], [1, 2], [T, 4]]),
                        pS[:], axis=AxX, op=Alu.add)

            if DEBUG:
                d_vEM = nc.declare_dram_parameter("d_vEM", [128, T * 64], f32, isOutput=True)
                d_sc = nc.declare_dram_parameter("d_sc", [128, T4], f32, isOutput=True)
                nc.sync.dma_start(
                    out=d_vEM[:], in_=AP(vEM[:], 0, [[T * 64, 128], [1, T * 64]]))
                nc.sync.dma_start(out=d_sc[:], in_=scoresF[:])

            # ---- segment softmax over (head, t) rows
            mrun = big.tile([128, T4], f32)
            nc.vector.tensor_tensor_scan(mrun[:], sb_m["sneg"][:], scoresF[:],
                                         NEG, op0=Alu.add, op1=Alu.max)
            em = big.tile([128, T4], f32)
            nc.vector.tensor_mul(em[:], mrun[:], sb_m["esel"][:])
            mmax = big.tile([128, T4], f32)

            def rev(t):
                return AP(t[:], T4 - 1, [[T4, 128], [-1, T4]])

            nc.vector.tensor_tensor_scan(rev(mmax), rev(sb_m["ekeep"]), rev(em),
                                         0.0, op0=Alu.mult, op1=Alu.add)
            ex = big.tile([128, T4], f32)
            nc.vector.tensor_sub(ex[:], scoresF[:], mmax[:])
            nc.scalar.activation(ex[:], ex[:], Act.Exp)
            nc.vector.tensor_mul(ex[:], ex[:], sb_m["pad"][:])
            if DEBUG:
                d_mm = nc.declare_dram_parameter("d_mm", [128, T4], f32, isOutput=True)
                d_ex = nc.declare_dram_parameter("d_ex", [128, T4], f32, isOutput=True)
                d_den = nc.declare_dram_parameter("d_den", [QROWS, 64], f32, isOutput=True)
                d_wv = nc.declare_dram_parameter("d_wv", [QROWS, 64], f32, isOutput=True)
                nc.sync.dma_start(out=d_mm[:], in_=mmax[:])
                nc.sync.dma_start(out=d_ex[:], in_=ex[:])

            # weighted v: vEM[p,t,f] *= ex[p, h(f), t]  (strided broadcast AP)
            nc.vector.tensor_mul(
                vEM[:], vEM[:],
                AP(ex[:], 0, [[T4, 128], [1, T], [T, 4], [0, 16]]))

            # segmented running sums (in place).  Each feature column is an
            # independent 2D strided scan; segment-start mask resets state.
            skeep = sb_m["skeep"]          # (128, 4T): 0 at starts, 1 inside
            sk1 = AP(skeep[:], 0, [[T4, 128], [1, T]])
            for fcol in range(64):
                col = AP(vEM[:], fcol, [[T * 64, 128], [64, T]])
                nc.vector.tensor_tensor_scan(col, sk1, col, 0.0,
                                             op0=Alu.mult, op1=Alu.add)
            for hcol in range(4):
                col = AP(ex[:], hcol * T, [[T4, 128], [1, T]])
                nc.vector.tensor_tensor_scan(col, sk1, col, 0.0,
                                             op0=Alu.mult, op1=Alu.add)
            # ex (scanned) replicated to (t, 64) for the denominator payload
            exr = big.tile([128, T, 64], f32, name="exr")
            nc.vector.tensor_copy(
                exr[:], AP(ex[:], 0, [[T4, 128], [1, T], [T, 4], [0, 16]]))

            # ---- output: every node's segment totals sit at its (static)
            # segment-end slot.  Bulk-copy [wv | den] to DRAM in flat slot
            # order, then a single node-major 512B-row gather (1 descriptor
            # per node instead of one scatter-add per slot), divide, store.
            nc.sync.dma_start(
                out=AP(vd_dram[:], 0, [[128, 128], [128 * 128, T], [1, 64]]),
                in_=vEM[:])
            nc.sync.dma_start(
                out=AP(vd_dram[:], 64, [[128, 128], [128 * 128, T], [1, 64]]),
                in_=exr[:])
            gout = big.tile([128, NCH, 128], f32, name="gout")
            qsplit = [(0, 3), (3, 6), (6, 8), (8, NCH)]
            for qn, (a, b) in enumerate(qsplit):
                nr = (b - a) * 128
                nc.gpsimd.dma_gather(
                    gout[:, a:b, :], vd_dram[:],
                    sb_iend[:, a * 8:b * 8], nr, nr, 128, queue_num=qn)
            den_sb = gout[:, :, 64:]
            wv_sb = gout[:, :, :64]
            nc.vector.tensor_scalar_max(den_sb, den_sb, 1e-30)
            nc.vector.reciprocal(den_sb, den_sb)
            nc.vector.tensor_mul(wv_sb, wv_sb, den_sb)
            nc.sync.dma_start(
                out=AP(out_d[:], 0, [[64, 128], [128 * 64, NCH], [1, 64]]),
                in_=wv_sb)

    nc.finalize()
    return nc


# ----------------------------------------------------------------------
# Entry point
# ----------------------------------------------------------------------

def kernel(b1, b2, edge_feats, f, src_idx, dst_idx,
           q_w, q_b,
           k_w1, k_b1, k_wl, k_bl, k_wr, k_br,
           v_w1, v_b1, v_wl, v_bl, v_wr, v_br,
           _run=None, _reps=1):
    kv = {"k": (k_w1, k_b1, k_wl, k_bl, k_wr, k_br),
          "v": (v_w1, v_b1, v_wl, v_bl, v_wr, v_br)}
    in_maps, T = _host_prep(b1, b2, edge_feats, f, src_idx, dst_idx,
                            q_w, q_b, kv)
    nc = build_program(T, reps=_reps)

    if _run is None:
        from concourse.bass_utils import run_bass_kernel_spmd
        res = run_bass_kernel_spmd(nc, in_maps, list(range(NCORES)))
        outs = [res.results[i]["out"] for i in range(NCORES)]
    else:
        outs = _run(nc, in_maps)

    full = np.concatenate([np.asarray(o)[:NPC] for o in outs], 0)
    return full.reshape(N, MULT, DIM).astype(np.float32)

